# revision 1
# baseline (speedup 1.0000x reference)
"""Trainium2 Bass kernel for a pre-LN transformer block (causal MHA + GELU FFN).

Problem: x[64, 512, 384], 7 heads x 54, FFN 2304. Sharded data-parallel over
batch across 8 NeuronCores (8 batches/core); no collectives needed.

Per-core dataflow (token-major <-> feature-major via PE transposes):
  LN1 (bn_stats)  -> xn tiles -> PE-transpose -> xnT [384, tok]
  QKV: Q^T/K^T feature-major per head-pair (partitions 0-53 / 64-117),
       V token-major with a ones-column per head (unnormalized-softmax trick)
  scores^T[t,s] per (b,h) -> exp (ACT, psum->sbuf) -> diag causal mask (DVE)
  o^T[55, s] = V_hat^T @ attn^T  (row 0 = softmax denominators)
  normalize = tensor_mul by gpsimd-broadcast reciprocal  (the psum->sbuf copy)
  att = O^T @ Wo (+x residual) -> x2 (DRAM bounce); LN2 -> xn2T; FFN1 ->
  gelu(+b1) -> FFN2 (two 9-chunk groups accumulating in one PSUM bank) ->
  out = x2 + ffn
All matmuls run as float32r (full PE rate at free-dim >= 256), fp32 accumulate.
"""

import numpy as np
from contextlib import ExitStack

import concourse.bass as bass
import concourse.bacc as bacc
import concourse.mybir as mybir
import concourse.tile as tile
from concourse import masks
from concourse.bass_utils import run_bass_kernel_spmd

# ---- problem constants (hardcoded per harness contract) ----
B, S, D = 64, 512, 384
H, HS = 7, 54
FFN = 6 * D  # 2304
EPS = 1e-5
N_CORES = 8
B_LOC = B // N_CORES          # 8 batches per core
F32 = mybir.dt.float32
F32R = mybir.dt.float32r
AF = mybir.ActivationFunctionType
ALU = mybir.AluOpType

N_D = D // 128                # 3 d-chunks
N_F = FFN // 128              # 18 ffn-chunks
PAIRS = (H + 1) // 2          # 4 head-pair groups (last has 1 head)

LAST_RESULTS = None


def _rr(ap):
    """bitcast an SBUF ap to float32r for full-rate PE streaming"""
    return ap.bitcast(F32R)


def to_fp32r(a):
    """Round fp32 to the fp32r format walrus expects for DMA'd matmul
    operands: 11-bit mantissa, RNE, low 12 bits zeroed."""
    u = np.ascontiguousarray(a, np.float32).view(np.uint32)
    r = (u + np.uint32(0x7FF) + ((u >> np.uint32(12)) & np.uint32(1))) & np.uint32(0xFFFFF000)
    return r.view(np.float32)


def build_program(n_b=B_LOC, has_bias_o=False, has_bias_2=False, mm_dt="f32r",
                  n_reps=1):
    nc = bacc.Bacc()
    NTOK = n_b * S
    NT = NTOK // 128          # token tiles per core

    cast = _rr if mm_dt == "f32r" else (lambda ap: ap)
    MDT = F32R if mm_dt == "f32r" else F32   # dtype of matmul-feeding tiles
    x_d = nc.declare_dram_parameter("x", [NTOK, D], F32, isOutput=False)
    wq_d = nc.declare_dram_parameter("wq_pad", [D, 512], MDT, isOutput=False)
    wk_d = nc.declare_dram_parameter("wk_pad", [D, 512], MDT, isOutput=False)
    wv_d = nc.declare_dram_parameter("wv_pad", [D, 512], MDT, isOutput=False)
    wo_d = nc.declare_dram_parameter("wo_pad", [H, 55, D], MDT, isOutput=False)
    w1_d = nc.declare_dram_parameter("w1", [D, FFN], MDT, isOutput=False)
    w2_d = nc.declare_dram_parameter("w2", [FFN, D], MDT, isOutput=False)
    b1_d = nc.declare_dram_parameter("b1c", [128, N_F], F32, isOutput=False)
    bo_d = nc.declare_dram_parameter("bo", [1, D], MDT, isOutput=False)
    b2_d = nc.declare_dram_parameter("b2", [1, D], MDT, isOutput=False)
    out_d = nc.declare_dram_parameter("out", [NTOK, D], F32, isOutput=True)

    with tile.TileContext(nc) as tc, ExitStack() as ctx:
        # ---------------- persistent pools ----------------
        wpool = ctx.enter_context(tc.tile_pool(name="weights", bufs=1))
        wq_sb = [wpool.tile([128, 512], MDT, tag=f"wq{d}", name=f"wq{d}") for d in range(N_D)]
        wk_sb = [wpool.tile([128, 512], MDT, tag=f"wk{d}", name=f"wk{d}") for d in range(N_D)]
        wv_sb = [wpool.tile([128, 512], MDT, tag=f"wv{d}", name=f"wv{d}") for d in range(N_D)]
        wo_sb = [wpool.tile([55, D], MDT, tag=f"wo{h}", name=f"wo{h}") for h in range(H)]
        w1_sb = [wpool.tile([128, FFN], MDT, tag=f"w1{d}", name=f"w1{d}") for d in range(N_D)]
        w2_sb = [wpool.tile([128, D], MDT, tag=f"w2{f}", name=f"w2{f}") for f in range(N_F)]
        b1_sb = wpool.tile([128, N_F], F32, tag="b1")
        bo_sb = wpool.tile([1, D], MDT, tag="bo")
        b2_sb = wpool.tile([1, D], MDT, tag="b2")
        eps_sb = wpool.tile([128, 1], F32, tag="eps")
        ones_sb = wpool.tile([1, 128], MDT, tag="ones")
        trimask = wpool.tile([128, 128], F32, tag="trimask")
        identity = wpool.tile([128, 128], F32, tag="ident")

        for d in range(N_D):
            nc.sync.dma_start(wq_sb[d][:], wq_d[128 * d:128 * (d + 1), :])
            nc.sync.dma_start(wk_sb[d][:], wk_d[128 * d:128 * (d + 1), :])
            nc.sync.dma_start(wv_sb[d][:], wv_d[128 * d:128 * (d + 1), :])
            nc.sync.dma_start(w1_sb[d][:], w1_d[128 * d:128 * (d + 1), :])
        for h in range(H):
            nc.sync.dma_start(wo_sb[h][:], wo_d[h])
        for f in range(N_F):
            nc.sync.dma_start(w2_sb[f][:], w2_d[128 * f:128 * (f + 1), :])
        nc.sync.dma_start(b1_sb[:], b1_d[:])
        nc.sync.dma_start(bo_sb[:], bo_d[:])
        nc.sync.dma_start(b2_sb[:], b2_d[:])
        nc.any.memset(eps_sb[:], EPS)
        nc.any.memset(ones_sb[:].bitcast(F32), 1.0)
        masks.make_identity(nc, identity[:])
        # trimask[t, s] = 1.0 if s >= t else 0.0  (upper triangular incl diag)
        masks.make_upper_triangular(nc, trimask[:], val=1.0, diag=True)

        # x2 bounce buffer in DRAM (SBUF is tight)
        dpool = ctx.enter_context(tc.tile_pool(name="dram", bufs=1, space="DRAM"))
        x2_ds = [dpool.tile([NTOK, D], F32, tag=f"x2d{r}", name=f"x2d{r}")
                 for r in range(n_reps)]
        chain = [dpool.tile([NTOK, D], F32, tag=f"chain{i}", name=f"chain{i}")
                 for i in range(max(n_reps - 1, 0))]

        # ---------------- streaming pools ----------------
        xpool = ctx.enter_context(tc.tile_pool(name="xin", bufs=4))
        stpool = ctx.enter_context(tc.tile_pool(name="stats", bufs=4))
        xnpool = ctx.enter_context(tc.tile_pool(name="xn", bufs=4))
        xTpool = ctx.enter_context(tc.tile_pool(name="xT", bufs=2))
        qkpool = ctx.enter_context(tc.tile_pool(name="qk", bufs=1))
        vpool = ctx.enter_context(tc.tile_pool(name="v", bufs=4))
        epool = ctx.enter_context(tc.tile_pool(name="expT", bufs=2))
        rpool = ctx.enter_context(tc.tile_pool(name="recip", bufs=2))
        otpool = ctx.enter_context(tc.tile_pool(name="oT", bufs=1))
        hpool = ctx.enter_context(tc.tile_pool(name="hgelu", bufs=10))
        opool = ctx.enter_context(tc.tile_pool(name="outt", bufs=4))

        ps_proj = ctx.enter_context(tc.tile_pool(name="ps_proj", bufs=3, space="PSUM"))
        ps_sc = ctx.enter_context(tc.tile_pool(name="ps_sc", bufs=1, space="PSUM"))
        ps_o = ctx.enter_context(tc.tile_pool(name="ps_o", bufs=1, space="PSUM"))

        def layernorm_tiles(src_tiles):
            """LN over 4 token tiles; returns 4 normalized tiles."""
            mv = stpool.tile([128, 8], F32, tag="mv", name="mv")
            for j in range(4):
                st6 = stpool.tile([128, 6], F32, tag="st6", name="st6")
                nc.vector.bn_stats(st6[:], src_tiles[j][:])
                nc.vector.bn_aggr(mv[:, 2 * j:2 * j + 2], st6[:])
            sg = stpool.tile([128, 4], F32, tag="sg", name="sg")
            rs = stpool.tile([128, 4], F32, tag="rs", name="rs")
            nmr = stpool.tile([128, 4], F32, tag="nmr", name="nmr")
            mv3 = mv[:].rearrange("p (j two) -> p j two", two=2)
            nc.scalar.activation(sg[:], mv3[:, :, 1], AF.Sqrt, bias=eps_sb[:, 0:1])
            nc.vector.reciprocal(rs[:], sg[:])
            for j in range(4):
                # nmr = -(mu * rsig)
                nc.vector.tensor_scalar(nmr[:, j:j + 1], mv3[:, j, 0].unsqueeze(-1),
                                        rs[:, j:j + 1], -1.0,
                                        op0=ALU.mult, op1=ALU.mult)
            xn_tiles = []
            for j in range(4):
                xn = xnpool.tile([128, D], F32, tag="xn", name="xn")
                nc.vector.tensor_scalar(xn[:], src_tiles[j][:], rs[:, j:j + 1],
                                        nmr[:, j:j + 1], op0=ALU.mult, op1=ALU.add)
                xn_tiles.append(xn)
            return xn_tiles

        def transpose_to_feature_major(xn_tiles):
            """4x [128, D] token-major -> 3x [128, 512] feature-major tiles."""
            xT = []
            for d in range(N_D):
                ps = ps_proj.tile([128, 512], F32, tag="ps", name="ps_t")
                for j in range(4):
                    nc.tensor.transpose(
                        ps[:, 128 * j:128 * (j + 1)],
                        xn_tiles[j][:, 128 * d:128 * (d + 1)],
                        identity[:],
                    )
                t = xTpool.tile([128, 512], MDT, tag=f"xT{d}", name=f"xT{d}")
                nc.any.tensor_copy(t[:], ps[:])
                xT.append(t)
            return xT

        # ======================= attention =======================
        for rep in range(n_reps):
          xsrc_d = x_d if rep == 0 else chain[rep - 1]
          xdst_d = out_d if rep == n_reps - 1 else chain[rep]
          x2_d = x2_ds[rep]
          for b in range(n_b):
            # ---- LN1 + transpose for this batch's 512 tokens ----
            xin = []
            for j in range(4):
                t0 = 128 * (4 * b + j)
                xt = xpool.tile([128, D], F32, tag="x", name="xt")
                nc.sync.dma_start(xt[:], xsrc_d[t0:t0 + 128, :])
                xin.append(xt)
            xn_tiles = layernorm_tiles(xin)
            xT = transpose_to_feature_major(xn_tiles)

            # ---- Q^T / K^T per head-pair: [54, 512] at partitions 0-53/64-117
            qt, kt = [], []
            for p in range(PAIRS):
                m = 118 if p < PAIRS - 1 else 54
                for (dst_list, w_sb, tg) in ((qt, wq_sb, "q"), (kt, wk_sb, "k")):
                    ps = ps_proj.tile([128, 512], F32, tag="ps", name="ps_qk")
                    for d in range(N_D):
                        nc.tensor.matmul(
                            ps[0:m, :],
                            cast(w_sb[d][:, 128 * p:128 * p + m]),
                            cast(xT[d][:]),
                            start=(d == 0), stop=(d == N_D - 1),
                        )
                    t = qkpool.tile([128, 512], MDT, tag=f"{tg}{p}", name=f"{tg}{p}")
                    nc.any.tensor_copy(t[0:m, :], ps[0:m, :])
                    dst_list.append(t)

            # ---- V token-major with ones column per head ----
            vt = []
            for j in range(4):
                ps = ps_proj.tile([128, 512], F32, tag="ps", name="ps_v")
                for d in range(N_D):
                    nc.tensor.matmul(
                        ps[:],
                        cast(xT[d][:, 128 * j:128 * (j + 1)]),
                        cast(wv_sb[d][:]),
                        start=(d == 0), stop=(d == N_D - 1),
                    )
                t = vpool.tile([128, 512], MDT, tag="v", name="vt")
                nc.any.memset(t[:].bitcast(F32), 1.0)
                src = ps[:, 0:448].rearrange("p (h c) -> p h c", h=H)[:, :, 1:55]
                dst = t[:, 0:448].rearrange("p (h c) -> p h c", h=H)[:, :, 1:55]
                nc.any.tensor_copy(dst, src)
                vt.append(t)

            # ---- per-head attention ----
            ot_b = [None] * H
            for h in range(H):
                p, sl = h // 2, 64 * (h % 2)
                sc = ps_sc.tile([128, 2048], F32, tag="sc", name="sc")
                for j in range(4):
                    # scores^T chunk j: [t=128, s in [128j, 512)]
                    nc.tensor.matmul(
                        sc[:, 512 * j + 128 * j: 512 * j + 512],
                        cast(kt[p][sl:sl + HS, 128 * j:128 * (j + 1)]),
                        cast(qt[p][sl:sl + HS, 128 * j:512]),
                        start=True, stop=True,
                    )
                eT = epool.tile([128, 2048], MDT, tag="eT", name="eT")
                for j in range(4):
                    lo, hi = 512 * j + 128 * j, 512 * j + 512
                    nc.scalar.activation(eT[:, lo:hi], sc[:, lo:hi], AF.Exp)
                # causal mask on the 4 diagonal blocks
                for j in range(4):
                    blk = eT[:, 640 * j: 640 * j + 128]
                    nc.vector.tensor_mul(blk, blk, trimask[:])
                # o^T accumulate over t-chunks; row 0 = softmax denominator
                ops = ps_o.tile([128, 512], F32, tag="o", name="ops")
                for j in range(4):
                    nc.tensor.matmul(
                        ops[0:55, 128 * j:512],
                        cast(vt[j][:, 64 * h: 64 * h + 55]),
                        cast(eT[:, 640 * j: 512 * j + 512]),
                        start=(j == 0), stop=(j == 3),
                    )
                r = rpool.tile([1, 512], F32, tag="r", name="r")
                rb = rpool.tile([55, 512], F32, tag="rb", name="rb")
                nc.vector.reciprocal_approx_fast(r[:], ops[0:1, :])
                nc.sync.dma_start(
                    rb[:], r[:].unsqueeze(1).to_broadcast([1, 55, 512]))
                ot = otpool.tile([55, 512], MDT, tag=f"ot{h}", name=f"ot{h}")
                nc.vector.tensor_mul(ot[:], ops[0:55, :], rb[:])
                ot_b[h] = ot

            # ---- attention out-proj + residual -> x2 (DRAM) ----
            for j in range(4):
                t0 = 128 * (4 * b + j)
                ps = ps_proj.tile([128, D], F32, tag="ps", name="ps_wo")
                for h in range(H):
                    nc.tensor.matmul(
                        ps[:],
                        cast(ot_b[h][:, 128 * j:128 * (j + 1)]),
                        cast(wo_sb[h][:]),
                        start=(h == 0), stop=(h == H - 1 and not has_bias_o),
                    )
                if has_bias_o:
                    nc.tensor.matmul(ps[:], cast(ones_sb[:]), cast(bo_sb[:]),
                                     start=False, stop=True)
                xr = xpool.tile([128, D], F32, tag="xr", name="xr")
                nc.sync.dma_start(xr[:], xsrc_d[t0:t0 + 128, :])
                x2t = opool.tile([128, D], F32, tag="out", name="x2t")
                nc.any.tensor_add(x2t[:], ps[:], xr[:])
                nc.sync.dma_start(x2_d[t0:t0 + 128, :], x2t[:])

          # ===================== FFN =====================
          for b in range(n_b):
            x2in = []
            for j in range(4):
                t0 = 128 * (4 * b + j)
                xt = xpool.tile([128, D], F32, tag="x", name="x2in")
                nc.sync.dma_start(xt[:], x2_d[t0:t0 + 128, :])
                x2in.append(xt)
            xn2_tiles = layernorm_tiles(x2in)
            xT2 = transpose_to_feature_major(xn2_tiles)

            # FFN2 accumulators: 4 tok-tiles share the 4 banks of one sc tile
            acc = ps_sc.tile([128, 2048], F32, tag="sc", name="acc")
            for g in range(2):
                hg = []
                for fi in range(9):
                    f = 9 * g + fi
                    ps = ps_proj.tile([128, 512], F32, tag="ps", name="ps_f1")
                    for d in range(N_D):
                        nc.tensor.matmul(
                            ps[:],
                            cast(w1_sb[d][:, 128 * f:128 * (f + 1)]),
                            cast(xT2[d][:]),
                            start=(d == 0), stop=(d == N_D - 1),
                        )
                    t = hpool.tile([128, 512], MDT, tag="hg", name="hg")
                    nc.scalar.activation(t[:], ps[:], AF.Gelu, bias=b1_sb[:, f:f + 1])
                    hg.append(t)
                for j in range(4):
                    for fi in range(9):
                        f = 9 * g + fi
                        nc.tensor.matmul(
                            acc[:, 512 * j: 512 * j + D],
                            cast(hg[fi][:, 128 * j:128 * (j + 1)]),
                            cast(w2_sb[f][:]),
                            start=(f == 0),
                            stop=(f == N_F - 1 and not has_bias_2),
                        )
            for j in range(4):
                t0 = 128 * (4 * b + j)
                if has_bias_2:
                    nc.tensor.matmul(acc[:, 512 * j:512 * j + D],
                                     cast(ones_sb[:]), cast(b2_sb[:]),
                                     start=False, stop=True)
                xr = xpool.tile([128, D], F32, tag="xr", name="xr2")
                nc.sync.dma_start(xr[:], x2_d[t0:t0 + 128, :])
                ot = opool.tile([128, D], F32, tag="out", name="outt")
                nc.any.tensor_add(ot[:], acc[:, 512 * j:512 * j + D], xr[:])
                nc.sync.dma_start(xdst_d[t0:t0 + 128, :], ot[:])

    nc.finalize()
    return nc


def preprocess(wq, bq, wk, bk, wv, bv, wo, bo, w1, b1, w2, b2,
               ln1_g, ln1_b, ln2_g, ln2_b):
    """Host-side folding: LN affine into weight matrices, attention scale into
    Q, V-bias into output bias; build padded/packed layouts."""
    f32 = np.float32
    args = [np.asarray(a, f32) for a in (wq, bq, wk, bk, wv, bv, wo, bo,
                                         w1, b1, w2, b2, ln1_g, ln1_b, ln2_g, ln2_b)]
    (wq, bq, wk, bk, wv, bv, wo, bo, w1, b1, w2, b2,
     ln1_g, ln1_b, ln2_g, ln2_b) = args
    scale = f32(HS) ** f32(-0.5)

    wq_pad = np.zeros((D, 512), f32)
    wk_pad = np.zeros((D, 512), f32)
    wv_pad = np.zeros((D, 512), f32)
    for h in range(H):
        wq_pad[:, 64 * h:64 * h + HS] = ln1_g[:, None] * wq[h] * scale
        wk_pad[:, 64 * h:64 * h + HS] = ln1_g[:, None] * wk[h]
        wv_pad[:, 64 * h + 1:64 * h + 1 + HS] = ln1_g[:, None] * wv[h]

    bq_eff = (bq + ln1_b @ wq).astype(f32)     # [H, HS]
    assert not np.any(bq_eff), "nonzero effective q bias not supported"
    # bk_eff shifts scores by a per-s constant -> cancelled by softmax; drop.

    bv_eff = (bv + ln1_b @ wv).astype(f32)     # [H, HS] -> folds into bo
    bo_eff = (bo + bv_eff.reshape(-1) @ wo).astype(f32)

    wo_pad = np.zeros((H, 55, D), f32)
    for h in range(H):
        wo_pad[h, 1:55, :] = wo[54 * h:54 * h + HS, :]

    w1_eff = (ln2_g[:, None] * w1).astype(f32)
    b1_eff = (b1 + ln2_b @ w1).astype(f32)
    b1c = np.ascontiguousarray(b1_eff.reshape(N_F, 128).T)   # [128, 18]

    return dict(
        wq_pad=to_fp32r(wq_pad), wk_pad=to_fp32r(wk_pad), wv_pad=to_fp32r(wv_pad),
        wo_pad=to_fp32r(wo_pad),
        w1=to_fp32r(w1_eff), b1c=b1c, w2=to_fp32r(w2),
        bo=to_fp32r(bo_eff.reshape(1, D)), b2=to_fp32r(b2.reshape(1, D)),
        has_bias_o=bool(np.any(bo_eff)), has_bias_2=bool(np.any(b2)),
    )


def kernel(**inputs):
    x = np.asarray(inputs["x"], np.float32)
    w = preprocess(
        inputs["wq"], inputs["bq"], inputs["wk"], inputs["bk"],
        inputs["wv"], inputs["bv"], inputs["wo"], inputs["bo"],
        inputs["w1"], inputs["b1"], inputs["w2"], inputs["b2"],
        inputs["ln1_g"], inputs["ln1_b"], inputs["ln2_g"], inputs["ln2_b"],
    )
    has_bo, has_b2 = w.pop("has_bias_o"), w.pop("has_bias_2")
    nc = build_program(n_b=B_LOC, has_bias_o=has_bo, has_bias_2=has_b2)

    core_ids = list(range(N_CORES))
    in_maps = []
    for c in core_ids:
        m = dict(w)
        m["x"] = np.ascontiguousarray(
            x[B_LOC * c:B_LOC * (c + 1)].reshape(B_LOC * S, D))
        in_maps.append(m)

    res = run_bass_kernel_spmd(nc, in_maps, core_ids)
    global LAST_RESULTS
    LAST_RESULTS = res
    out = np.concatenate(
        [res.results[i]["out"].reshape(B_LOC, S, D) for i in range(N_CORES)], axis=0
    )
    return out.astype(np.float32)



# revision 7
# speedup vs baseline: 1.1482x; 1.1482x over previous
"""Trainium2 Bass kernel for a pre-LN transformer block (causal MHA + GELU FFN).

Problem: x[64, 512, 384], 7 heads x 54, FFN 2304. Sharded data-parallel over
batch across 8 NeuronCores (8 batches/core); no collectives needed.

v1 (bf16 rewrite of the fp32r baseline):
  - all matmul operands bf16 (fp32 accumulate in PSUM); rel-err budget 2e-2
  - LN rsqrt via exp(-0.5*ln(var+eps)) so the ACT engine table set stays on
    {ln,exp} for the whole attention phase and {gelu} for the FFN phase
    (2 table loads per rep instead of ~3 per batch)
  - per-head scores live in a 2-bank bf16 PSUM tile packed causally:
    4 diagonal 128-blocks at cols 0..512, rectangles at 512..896 /
    1024..1280 / 1280..1408; one exp() call [0..1408]; one 4-block
    triangular mask multiply on the diagonal region
  - x2 (post-attention residual) persists in SBUF; x is loaded once
  - FFN2 accumulates into the (phase-A scores) PSUM tiles bitcast to f32
"""

import numpy as np
from contextlib import ExitStack

import concourse.bass as bass
import concourse.bacc as bacc
import concourse.mybir as mybir
import concourse.tile as tile
from concourse import masks
from concourse.bass_utils import run_bass_kernel_spmd

# ---- problem constants (hardcoded per harness contract) ----
B, S, D = 64, 512, 384
H, HS = 7, 54
FFN = 6 * D  # 2304
EPS = 1e-5
N_CORES = 8
B_LOC = B // N_CORES          # 8 batches per core
F32 = mybir.dt.float32
BF16 = mybir.dt.bfloat16
AF = mybir.ActivationFunctionType
ALU = mybir.AluOpType

N_D = D // 128                # 3 d-chunks
N_F = FFN // 128              # 18 ffn-chunks
PAIRS = (H + 1) // 2          # 4 head-pair groups (last has 1 head)

# packed causal-score layout inside a [128, 1536] f32 PSUM tile (3 banks;
# f32 bank = 512 elements -> matmul outputs must not cross 512-col lines):
# bank0: 4 diagonal 128-blocks at cols 128j; bank1: rect j=0 (384 w) +
# rect j=2 (128 w); bank2: rect j=1 (256 w)
DIAG_OFF = 0
R_OFF = (512, 1024, 896)      # rect offsets for j=0,1,2
R_W = (384, 256, 128)
EXP_END = 1280                # exp() covers [0, 1280) in one call

LAST_RESULTS = None


def build_program(n_b=B_LOC, has_bias_o=False, has_bias_2=False, n_reps=1):
    nc = bacc.Bacc()
    NTOK = n_b * S

    x_d = nc.declare_dram_parameter("x", [NTOK, D], F32, isOutput=False)
    wq_d = nc.declare_dram_parameter("wq_pad", [D, 512], BF16, isOutput=False)
    wk_d = nc.declare_dram_parameter("wk_pad", [D, 512], BF16, isOutput=False)
    wv_d = nc.declare_dram_parameter("wv_pad", [D, 512], BF16, isOutput=False)
    wo_d = nc.declare_dram_parameter("wo_pad", [H, 55, D], BF16, isOutput=False)
    w1_d = nc.declare_dram_parameter("w1", [D, FFN], BF16, isOutput=False)
    w2_d = nc.declare_dram_parameter("w2", [FFN, D], BF16, isOutput=False)
    b1_d = nc.declare_dram_parameter("b1c", [128, N_F], F32, isOutput=False)
    bo_d = nc.declare_dram_parameter("bo", [1, D], BF16, isOutput=False)
    b2_d = nc.declare_dram_parameter("b2", [1, D], BF16, isOutput=False)
    out_d = nc.declare_dram_parameter("out", [NTOK, D], F32, isOutput=True)

    with tile.TileContext(nc) as tc, ExitStack() as ctx, \
            nc.allow_low_precision(reason="bf16 kernel; rel-err gate 2e-2"):
        # ---------------- persistent pools ----------------
        wpool = ctx.enter_context(tc.tile_pool(name="weights", bufs=1))
        wq_sb = [wpool.tile([128, 512], BF16, tag=f"wq{d}", name=f"wq{d}") for d in range(N_D)]
        wk_sb = [wpool.tile([128, 512], BF16, tag=f"wk{d}", name=f"wk{d}") for d in range(N_D)]
        wv_sb = [wpool.tile([128, 512], BF16, tag=f"wv{d}", name=f"wv{d}") for d in range(N_D)]
        wo_sb = [wpool.tile([55, D], BF16, tag=f"wo{h}", name=f"wo{h}") for h in range(H)]
        w1_sb = [wpool.tile([128, FFN], BF16, tag=f"w1{d}", name=f"w1{d}") for d in range(N_D)]
        w2_sb = [wpool.tile([128, D], BF16, tag=f"w2{f}", name=f"w2{f}") for f in range(N_F)]
        b1_sb = wpool.tile([128, N_F], F32, tag="b1")
        bo_sb = wpool.tile([1, D], BF16, tag="bo")
        b2_sb = wpool.tile([1, D], BF16, tag="b2")
        eps_sb = wpool.tile([128, 1], F32, tag="eps")
        ones_sb = wpool.tile([1, 128], BF16, tag="ones")
        tri4 = wpool.tile([128, 512], BF16, tag="tri4")
        identity = wpool.tile([128, 128], BF16, tag="ident")

        # x2 persists in SBUF between the attention and FFN phases
        x2pool = ctx.enter_context(tc.tile_pool(name="x2", bufs=1))
        x2_sb = [[x2pool.tile([128, D], F32, tag=f"x2_{b}_{j}", name=f"x2_{b}_{j}")
                  for j in range(4)] for b in range(n_b)]
        ln2_rs = [x2pool.tile([128, 4], F32, tag=f"rs2_{b}", name=f"rs2_{b}") for b in range(n_b)]
        ln2_nm = [x2pool.tile([128, 4], F32, tag=f"nm2_{b}", name=f"nm2_{b}") for b in range(n_b)]

        # prologue weight DMAs (first wave: everything phase A needs)
        for d in range(N_D):
            nc.sync.dma_start(wq_sb[d][:], wq_d[128 * d:128 * (d + 1), :])
            nc.sync.dma_start(wk_sb[d][:], wk_d[128 * d:128 * (d + 1), :])
            nc.sync.dma_start(wv_sb[d][:], wv_d[128 * d:128 * (d + 1), :])
        for h in range(H):
            nc.sync.dma_start(wo_sb[h][:], wo_d[h])
        nc.sync.dma_start(b1_sb[:], b1_d[:])
        nc.sync.dma_start(bo_sb[:], bo_d[:])
        nc.sync.dma_start(b2_sb[:], b2_d[:])
        nc.any.memset(eps_sb[:], EPS)
        nc.any.memset(ones_sb[:], 1.0)
        masks.make_identity(nc, identity[:])
        for j in range(4):
            masks.make_upper_triangular(nc, tri4[:, 128 * j:128 * (j + 1)],
                                        val=1.0, diag=True)

        # second wave: FFN weights (queued behind phase-A essentials)
        for d in range(N_D):
            nc.sync.dma_start(w1_sb[d][:], w1_d[128 * d:128 * (d + 1), :])
        for f in range(N_F):
            nc.sync.dma_start(w2_sb[f][:], w2_d[128 * f:128 * (f + 1), :])

        dpool = ctx.enter_context(tc.tile_pool(name="dram", bufs=1, space="DRAM"))
        chain = [dpool.tile([NTOK, D], F32, tag=f"chain{i}", name=f"chain{i}")
                 for i in range(max(n_reps - 1, 0))]

        # ---------------- streaming pools ----------------
        xpool = ctx.enter_context(tc.tile_pool(name="xin", bufs=8))
        stpool = ctx.enter_context(tc.tile_pool(name="stats", bufs=4))
        xnpool = ctx.enter_context(tc.tile_pool(name="xn", bufs=8))
        xTpool = ctx.enter_context(tc.tile_pool(name="xT", bufs=2))
        qkpool = ctx.enter_context(tc.tile_pool(name="qk", bufs=2))
        vpool = ctx.enter_context(tc.tile_pool(name="v", bufs=8))
        epool = ctx.enter_context(tc.tile_pool(name="expT", bufs=2))
        rpool = ctx.enter_context(tc.tile_pool(name="recip", bufs=2))
        otpool = ctx.enter_context(tc.tile_pool(name="oT", bufs=2))
        hpool = ctx.enter_context(tc.tile_pool(name="hgelu", bufs=4))
        opool = ctx.enter_context(tc.tile_pool(name="outt", bufs=4))

        # PSUM budget (8 banks): sc 2x3 + shared proj/o 2x1
        ps_sc = ctx.enter_context(tc.tile_pool(name="ps_sc", bufs=2, space="PSUM"))
        ps_po = ctx.enter_context(tc.tile_pool(name="ps_po", bufs=2, space="PSUM"))

        def layernorm_tiles(src_tiles, rs_out=None, nm_out=None):
            """LN over 4 token tiles; returns 4 normalized bf16 tiles.
            rsqrt computed as exp(-0.5*ln(var+eps)) to stay on ACT set
            {ln,exp}. If rs_out/nm_out given, only computes the scales
            (phase-A LN2 stats) and returns None."""
            mv = stpool.tile([128, 8], F32, tag="mv", name="mv")
            for j in range(4):
                st6 = stpool.tile([128, 6], F32, tag="st6", name="st6")
                nc.vector.bn_stats(st6[:], src_tiles[j][:])
                nc.vector.bn_aggr(mv[:, 2 * j:2 * j + 2], st6[:])
            lnv = stpool.tile([128, 4], F32, tag="lnv", name="lnv")
            rs = rs_out if rs_out is not None else stpool.tile([128, 4], F32, tag="rs", name="rs")
            nmr = nm_out if nm_out is not None else stpool.tile([128, 4], F32, tag="nmr", name="nmr")
            mv3 = mv[:].rearrange("p (j two) -> p j two", two=2)
            nc.scalar.activation(lnv[:], mv3[:, :, 1], AF.Ln, bias=eps_sb[:, 0:1])
            nc.scalar.activation(rs[:], lnv[:], AF.Exp, scale=-0.5)
            for j in range(4):
                # nmr = -(mu * rsig)
                nc.vector.tensor_scalar(nmr[:, j:j + 1], mv3[:, j, 0].unsqueeze(-1),
                                        rs[:, j:j + 1], -1.0,
                                        op0=ALU.mult, op1=ALU.mult)
            if rs_out is not None:
                return None
            xn_tiles = []
            for j in range(4):
                xn = xnpool.tile([128, D], BF16, tag="xn", name="xn")
                nc.vector.tensor_scalar(xn[:], src_tiles[j][:], rs[:, j:j + 1],
                                        nmr[:, j:j + 1], op0=ALU.mult, op1=ALU.add)
                xn_tiles.append(xn)
            return xn_tiles

        def apply_ln(src_tiles, rs, nmr):
            xn_tiles = []
            for j in range(4):
                xn = xnpool.tile([128, D], BF16, tag="xn", name="xn")
                nc.vector.tensor_scalar(xn[:], src_tiles[j][:], rs[:, j:j + 1],
                                        nmr[:, j:j + 1], op0=ALU.mult, op1=ALU.add)
                xn_tiles.append(xn)
            return xn_tiles

        def transpose_to_feature_major(xn_tiles):
            """4x [128, D] token-major bf16 -> 3x [128, 512] feature-major."""
            xT = []
            for d in range(N_D):
                ps = ps_po.tile([128, 512], F32, tag="po", name="ps_t")
                psb = ps[:].bitcast(BF16)   # transpose out must match in dtype
                for j in range(4):
                    nc.tensor.transpose(
                        psb[:, 128 * j:128 * (j + 1)],
                        xn_tiles[j][:, 128 * d:128 * (d + 1)],
                        identity[:],
                    )
                t = xTpool.tile([128, 512], BF16, tag=f"xT{d}", name=f"xT{d}")
                nc.vector.tensor_copy(t[:], psb[:, 0:512])
                xT.append(t)
            return xT

        # ======================= per-rep =======================
        for rep in range(n_reps):
          xsrc_d = x_d if rep == 0 else chain[rep - 1]
          xdst_d = out_d if rep == n_reps - 1 else chain[rep]

          # ------------------- phase A: attention -------------------
          for b in range(n_b):
            xin = []
            for j in range(4):
                t0 = 128 * (4 * b + j)
                xt = xpool.tile([128, D], F32, tag="x", name="xt")
                nc.sync.dma_start(xt[:], xsrc_d[t0:t0 + 128, :])
                xin.append(xt)
            xn_tiles = layernorm_tiles(xin)
            xT = transpose_to_feature_major(xn_tiles)

            # Q^T / K^T per head-pair: [54, 512] at partitions 0-53/64-117
            qt, kt = [], []
            for p in range(PAIRS):
                m = 118 if p < PAIRS - 1 else 54
                for (dst_list, w_sb, tg) in ((qt, wq_sb, "q"), (kt, wk_sb, "k")):
                    ps = ps_po.tile([128, 512], F32, tag="po", name="ps_qk")
                    for d in range(N_D):
                        nc.tensor.matmul(
                            ps[0:m, :],
                            w_sb[d][:, 128 * p:128 * p + m],
                            xT[d][:],
                            start=(d == 0), stop=(d == N_D - 1),
                        )
                    t = qkpool.tile([128, 512], BF16, tag=f"{tg}{p}", name=f"{tg}{p}")
                    nc.scalar.copy(t[0:m, :], ps[0:m, :])
                    dst_list.append(t)

            # V token-major with a ones column per head
            vt = []
            for j in range(4):
                ps = ps_po.tile([128, 512], F32, tag="po", name="ps_v")
                for d in range(N_D):
                    nc.tensor.matmul(
                        ps[:],
                        xT[d][:, 128 * j:128 * (j + 1)],
                        wv_sb[d][:],
                        start=(d == 0), stop=(d == N_D - 1),
                    )
                t = vpool.tile([128, 512], BF16, tag="v", name="vt")
                nc.gpsimd.memset(t[:], 1.0)
                src = ps[:, 0:448].rearrange("p (h c) -> p h c", h=H)[:, :, 1:55]
                dst = t[:, 0:448].rearrange("p (h c) -> p h c", h=H)[:, :, 1:55]
                nc.scalar.copy(dst, src)
                vt.append(t)

            # ---- per-head attention ----
            ot_b = [None] * H
            for h in range(H):
                p, sl = h // 2, 64 * (h % 2)
                sc = ps_sc.tile([128, 1536], F32, tag="sc", name="sc")
                for j in range(4):
                    # shared stationary kt chunk: diagonal block then rect
                    nc.tensor.matmul(
                        sc[:, 128 * j:128 * (j + 1)],
                        kt[p][sl:sl + HS, 128 * j:128 * (j + 1)],
                        qt[p][sl:sl + HS, 128 * j:128 * (j + 1)],
                        start=True, stop=True,
                    )
                    if j < 3:
                        nc.tensor.matmul(
                            sc[:, R_OFF[j]:R_OFF[j] + R_W[j]],
                            kt[p][sl:sl + HS, 128 * j:128 * (j + 1)],
                            qt[p][sl:sl + HS, 128 * (j + 1):512],
                            start=True, stop=True,
                        )
                eT = epool.tile([128, 1280], BF16, tag="eT", name="eT")
                nc.scalar.activation(eT[:, 0:EXP_END], sc[:, 0:EXP_END], AF.Exp)
                # causal mask on the packed diagonal blocks (one DVE op)
                nc.vector.tensor_mul(eT[:, 0:512], eT[:, 0:512], tri4[:])
                # o^T accumulate; row 0 = softmax denominator
                ops = ps_po.tile([128, 512], F32, tag="po", name="ops")
                for j in range(3):
                    nc.tensor.matmul(
                        ops[0:55, 128 * (j + 1):512],
                        vt[j][:, 64 * h: 64 * h + 55],
                        eT[:, R_OFF[j]:R_OFF[j] + R_W[j]],
                        start=(j == 0), stop=False,
                    )
                for j in range(4):
                    nc.tensor.matmul(
                        ops[0:55, 128 * j:128 * (j + 1)],
                        vt[j][:, 64 * h: 64 * h + 55],
                        eT[:, 128 * j:128 * (j + 1)],
                        start=False, stop=(j == 3),
                    )
                osb = rpool.tile([55, 512], BF16, tag="osb", name="osb")
                nc.vector.tensor_copy(osb[:], ops[0:55, :])
                r = rpool.tile([1, 512], BF16, tag="r", name="r")
                nc.vector.reciprocal(r[:], ops[0:1, :])
                rb = rpool.tile([55, 512], BF16, tag="rb", name="rb")
                nc.sync.dma_start(
                    rb[:], r[:].unsqueeze(1).to_broadcast([1, 55, 512]))
                ot = otpool.tile([55, 512], BF16, tag=f"ot{h}", name=f"ot{h}")
                nc.vector.tensor_mul(ot[:], osb[:], rb[:])
                ot_b[h] = ot

            # ---- attention out-proj + residual -> x2 (SBUF) ----
            for j in range(4):
                ps = ps_po.tile([128, 512], F32, tag="po", name="ps_wo")
                for h in range(H):
                    nc.tensor.matmul(
                        ps[:, 0:D],
                        ot_b[h][:, 128 * j:128 * (j + 1)],
                        wo_sb[h][:],
                        start=(h == 0), stop=(h == H - 1 and not has_bias_o),
                    )
                if has_bias_o:
                    nc.tensor.matmul(ps[:, 0:D], ones_sb[:], bo_sb[:],
                                     start=False, stop=True)
                nc.vector.tensor_add(x2_sb[b][j][:], ps[:, 0:D], xin[j][:])
            # LN2 stats for this batch (ACT still on {ln,exp})
            layernorm_tiles([x2_sb[b][j] for j in range(4)],
                            rs_out=ln2_rs[b], nm_out=ln2_nm[b])

          # ------------------- phase B: FFN -------------------
          for b in range(n_b):
            xn2 = apply_ln([x2_sb[b][j] for j in range(4)],
                           ln2_rs[b], ln2_nm[b])
            xT2 = transpose_to_feature_major(xn2)

            # FFN2 accumulators: 4 token-chunks in the 2 sc psum tiles
            # (bitcast to f32: [128, 1024] = 2 banks; chunks at 0 and 512)
            acc_t = [ps_sc.tile([128, 1536], F32, tag="sc", name="acc")
                     for _ in range(2)]
            acc = [acc_t[j // 2][:, 512 * (j % 2): 512 * (j % 2) + D]
                   for j in range(4)]
            for f in range(N_F):
                ps = ps_po.tile([128, 512], F32, tag="po", name="ps_f1")
                for d in range(N_D):
                    nc.tensor.matmul(
                        ps[:],
                        w1_sb[d][:, 128 * f:128 * (f + 1)],
                        xT2[d][:],
                        start=(d == 0), stop=(d == N_D - 1),
                    )
                hg = hpool.tile([128, 512], BF16, tag="hg", name="hg")
                nc.scalar.activation(hg[:], ps[:], AF.Gelu, bias=b1_sb[:, f:f + 1])
                for j in range(4):
                    nc.tensor.matmul(
                        acc[j],
                        hg[:, 128 * j:128 * (j + 1)],
                        w2_sb[f][:],
                        start=(f == 0),
                        stop=(f == N_F - 1 and not has_bias_2),
                    )
            for j in range(4):
                t0 = 128 * (4 * b + j)
                if has_bias_2:
                    nc.tensor.matmul(acc[j], ones_sb[:], b2_sb[:],
                                     start=False, stop=True)
                ot = opool.tile([128, D], F32, tag="out", name="outt")
                nc.vector.tensor_add(ot[:], acc[j], x2_sb[b][j][:])
                nc.sync.dma_start(xdst_d[t0:t0 + 128, :], ot[:])

    nc.finalize()
    return nc


def preprocess(wq, bq, wk, bk, wv, bv, wo, bo, w1, b1, w2, b2,
               ln1_g, ln1_b, ln2_g, ln2_b):
    """Host-side folding: LN affine into weight matrices, attention scale into
    Q, V-bias into output bias; build padded/packed bf16 layouts."""
    import ml_dtypes
    f32 = np.float32
    bf16 = ml_dtypes.bfloat16
    args = [np.asarray(a, f32) for a in (wq, bq, wk, bk, wv, bv, wo, bo,
                                         w1, b1, w2, b2, ln1_g, ln1_b, ln2_g, ln2_b)]
    (wq, bq, wk, bk, wv, bv, wo, bo, w1, b1, w2, b2,
     ln1_g, ln1_b, ln2_g, ln2_b) = args
    scale = f32(HS) ** f32(-0.5)

    wq_pad = np.zeros((D, 512), f32)
    wk_pad = np.zeros((D, 512), f32)
    wv_pad = np.zeros((D, 512), f32)
    for h in range(H):
        wq_pad[:, 64 * h:64 * h + HS] = ln1_g[:, None] * wq[h] * scale
        wk_pad[:, 64 * h:64 * h + HS] = ln1_g[:, None] * wk[h]
        wv_pad[:, 64 * h + 1:64 * h + 1 + HS] = ln1_g[:, None] * wv[h]

    bq_eff = (bq + ln1_b @ wq).astype(f32)     # [H, HS]
    assert not np.any(bq_eff), "nonzero effective q bias not supported"
    # bk_eff shifts scores by a per-s constant -> cancelled by softmax; drop.

    bv_eff = (bv + ln1_b @ wv).astype(f32)     # [H, HS] -> folds into bo
    bo_eff = (bo + bv_eff.reshape(-1) @ wo).astype(f32)

    wo_pad = np.zeros((H, 55, D), f32)
    for h in range(H):
        wo_pad[h, 1:55, :] = wo[54 * h:54 * h + HS, :]

    w1_eff = (ln2_g[:, None] * w1).astype(f32)
    b1_eff = (b1 + ln2_b @ w1).astype(f32)
    b1c = np.ascontiguousarray(b1_eff.reshape(N_F, 128).T)   # [128, 18]

    def bf(a):
        return np.ascontiguousarray(a).astype(bf16)

    return dict(
        wq_pad=bf(wq_pad), wk_pad=bf(wk_pad), wv_pad=bf(wv_pad),
        wo_pad=bf(wo_pad),
        w1=bf(w1_eff), b1c=b1c, w2=bf(w2),
        bo=bf(bo_eff.reshape(1, D)), b2=bf(b2.reshape(1, D)),
        has_bias_o=bool(np.any(bo_eff)), has_bias_2=bool(np.any(b2)),
    )


def kernel(**inputs):
    x = np.asarray(inputs["x"], np.float32)
    w = preprocess(
        inputs["wq"], inputs["bq"], inputs["wk"], inputs["bk"],
        inputs["wv"], inputs["bv"], inputs["wo"], inputs["bo"],
        inputs["w1"], inputs["b1"], inputs["w2"], inputs["b2"],
        inputs["ln1_g"], inputs["ln1_b"], inputs["ln2_g"], inputs["ln2_b"],
    )
    has_bo, has_b2 = w.pop("has_bias_o"), w.pop("has_bias_2")
    nc = build_program(n_b=B_LOC, has_bias_o=has_bo, has_bias_2=has_b2)

    core_ids = list(range(N_CORES))
    in_maps = []
    for c in core_ids:
        m = dict(w)
        m["x"] = np.ascontiguousarray(
            x[B_LOC * c:B_LOC * (c + 1)].reshape(B_LOC * S, D))
        in_maps.append(m)

    res = run_bass_kernel_spmd(nc, in_maps, core_ids)
    global LAST_RESULTS
    LAST_RESULTS = res
    out = np.concatenate(
        [res.results[i]["out"].reshape(B_LOC, S, D) for i in range(N_CORES)], axis=0
    )
    return out.astype(np.float32)


# revision 9
# speedup vs baseline: 1.3007x; 1.1329x over previous
"""Trainium2 Bass kernel for a pre-LN transformer block (causal MHA + GELU FFN).

Problem: x[64, 512, 384], 7 heads x 54, FFN 2304. Sharded data-parallel over
batch across 8 NeuronCores (8 batches/core); no collectives needed.

v1 (bf16 rewrite of the fp32r baseline):
  - all matmul operands bf16 (fp32 accumulate in PSUM); rel-err budget 2e-2
  - LN rsqrt via exp(-0.5*ln(var+eps)) so the ACT engine table set stays on
    {ln,exp} for the whole attention phase and {gelu} for the FFN phase
    (2 table loads per rep instead of ~3 per batch)
  - per-head scores live in a 2-bank bf16 PSUM tile packed causally:
    4 diagonal 128-blocks at cols 0..512, rectangles at 512..896 /
    1024..1280 / 1280..1408; one exp() call [0..1408]; one 4-block
    triangular mask multiply on the diagonal region
  - x2 (post-attention residual) persists in SBUF; x is loaded once
  - FFN2 accumulates into the (phase-A scores) PSUM tiles bitcast to f32
"""

import numpy as np
from contextlib import ExitStack

import concourse.bass as bass
import concourse.bacc as bacc
import concourse.mybir as mybir
import concourse.tile as tile
from concourse import masks
from concourse.bass_utils import run_bass_kernel_spmd

# ---- problem constants (hardcoded per harness contract) ----
B, S, D = 64, 512, 384
H, HS = 7, 54
FFN = 6 * D  # 2304
EPS = 1e-5
N_CORES = 8
B_LOC = B // N_CORES          # 8 batches per core
F32 = mybir.dt.float32
BF16 = mybir.dt.bfloat16
AF = mybir.ActivationFunctionType
ALU = mybir.AluOpType

N_D = D // 128                # 3 d-chunks
N_F = FFN // 128              # 18 ffn-chunks
PAIRS = (H + 1) // 2          # 4 head-pair groups (last has 1 head)

# packed causal-score layout inside a [128, 1536] f32 PSUM tile (3 banks;
# f32 bank = 512 elements -> matmul outputs must not cross 512-col lines):
# bank0: 4 diagonal 128-blocks at cols 128j; bank1: rect j=0 (384 w) +
# rect j=2 (128 w); bank2: rect j=1 (256 w)
DIAG_OFF = 0
R_OFF = (512, 1024, 896)      # rect offsets for j=0,1,2
R_W = (384, 256, 128)
EXP_END = 1280                # exp() covers [0, 1280) in one call

LAST_RESULTS = None


def build_program(n_b=B_LOC, has_bias_o=False, has_bias_2=False, n_reps=1):
    nc = bacc.Bacc()
    NTOK = n_b * S

    x_d = nc.declare_dram_parameter("x", [NTOK, D], F32, isOutput=False)
    wq_d = nc.declare_dram_parameter("wq_pad", [D, 512], BF16, isOutput=False)
    wk_d = nc.declare_dram_parameter("wk_pad", [D, 512], BF16, isOutput=False)
    wv_d = nc.declare_dram_parameter("wv_pad", [D, 512], BF16, isOutput=False)
    wo_d = nc.declare_dram_parameter("wo_pad", [H, 55, D], BF16, isOutput=False)
    w1_d = nc.declare_dram_parameter("w1", [D, FFN], BF16, isOutput=False)
    w2_d = nc.declare_dram_parameter("w2", [FFN, D], BF16, isOutput=False)
    b1_d = nc.declare_dram_parameter("b1c", [128, N_F], F32, isOutput=False)
    bo_d = nc.declare_dram_parameter("bo", [1, D], BF16, isOutput=False)
    b2_d = nc.declare_dram_parameter("b2", [1, D], BF16, isOutput=False)
    out_d = nc.declare_dram_parameter("out", [NTOK, D], F32, isOutput=True)

    with tile.TileContext(nc) as tc, ExitStack() as ctx, \
            nc.allow_low_precision(reason="bf16 kernel; rel-err gate 2e-2"):
        # ---------------- persistent pools ----------------
        wpool = ctx.enter_context(tc.tile_pool(name="weights", bufs=1))
        wq_sb = [wpool.tile([128, 512], BF16, tag=f"wq{d}", name=f"wq{d}") for d in range(N_D)]
        wk_sb = [wpool.tile([128, 512], BF16, tag=f"wk{d}", name=f"wk{d}") for d in range(N_D)]
        wv_sb = [wpool.tile([128, 512], BF16, tag=f"wv{d}", name=f"wv{d}") for d in range(N_D)]
        wo_sb = [wpool.tile([55, D], BF16, tag=f"wo{h}", name=f"wo{h}") for h in range(H)]
        w1_sb = [wpool.tile([128, FFN], BF16, tag=f"w1{d}", name=f"w1{d}") for d in range(N_D)]
        w2_sb = [wpool.tile([128, D], BF16, tag=f"w2{f}", name=f"w2{f}") for f in range(N_F)]
        b1_sb = wpool.tile([128, N_F], F32, tag="b1")
        bo_sb = wpool.tile([1, D], BF16, tag="bo")
        b2_sb = wpool.tile([1, D], BF16, tag="b2")
        eps_sb = wpool.tile([128, 1], F32, tag="eps")
        magic_sb = wpool.tile([128, 4], mybir.dt.uint32, tag="magic")
        ones_sb = wpool.tile([1, 128], BF16, tag="ones")
        tri4 = wpool.tile([128, 512], BF16, tag="tri4")
        identity = wpool.tile([128, 128], BF16, tag="ident")

        # x2 persists in SBUF between the attention and FFN phases
        x2pool = ctx.enter_context(tc.tile_pool(name="x2", bufs=1))
        x2_sb = [[x2pool.tile([128, D], F32, tag=f"x2_{b}_{j}", name=f"x2_{b}_{j}")
                  for j in range(4)] for b in range(n_b)]
        ln2_rs = [x2pool.tile([128, 4], F32, tag=f"rs2_{b}", name=f"rs2_{b}") for b in range(n_b)]
        ln2_nm = [x2pool.tile([128, 4], F32, tag=f"nm2_{b}", name=f"nm2_{b}") for b in range(n_b)]

        # prologue weight DMAs (first wave: everything phase A needs)
        for d in range(N_D):
            nc.sync.dma_start(wq_sb[d][:], wq_d[128 * d:128 * (d + 1), :])
            nc.sync.dma_start(wk_sb[d][:], wk_d[128 * d:128 * (d + 1), :])
            nc.sync.dma_start(wv_sb[d][:], wv_d[128 * d:128 * (d + 1), :])
        for h in range(H):
            nc.sync.dma_start(wo_sb[h][:], wo_d[h])
        nc.sync.dma_start(b1_sb[:], b1_d[:])
        nc.sync.dma_start(bo_sb[:], bo_d[:])
        nc.sync.dma_start(b2_sb[:], b2_d[:])
        nc.any.memset(eps_sb[:], EPS)
        nc.any.memset(magic_sb[:].bitcast(F32), np.uint32(0x5F3759DF).view(np.float32))
        nc.any.memset(ones_sb[:], 1.0)
        masks.make_identity(nc, identity[:])
        for j in range(4):
            masks.make_upper_triangular(nc, tri4[:, 128 * j:128 * (j + 1)],
                                        val=1.0, diag=True)

        # ---------------- streaming pools ----------------
        xpool = ctx.enter_context(tc.tile_pool(name="xin", bufs=8))
        stpool = ctx.enter_context(tc.tile_pool(name="stats", bufs=4))
        xnpool = ctx.enter_context(tc.tile_pool(name="xn", bufs=8))
        xTpool = ctx.enter_context(tc.tile_pool(name="xT", bufs=2))
        qkpool = ctx.enter_context(tc.tile_pool(name="qk", bufs=2))
        vpool = ctx.enter_context(tc.tile_pool(name="v", bufs=8))
        epool = ctx.enter_context(tc.tile_pool(name="expT", bufs=2))
        rpool = ctx.enter_context(tc.tile_pool(name="recip", bufs=2))
        otpool = ctx.enter_context(tc.tile_pool(name="oT", bufs=2))
        hpool = ctx.enter_context(tc.tile_pool(name="hgelu", bufs=4))
        opool = ctx.enter_context(tc.tile_pool(name="outt", bufs=4))

        # prefetch x for the first two batches ahead of the FFN weights
        xpre = {}
        for b in range(min(2, n_b)):
            for j in range(4):
                t0 = 128 * (4 * b + j)
                xt = xpool.tile([128, D], F32, tag="x", name="xt")
                nc.sync.dma_start(xt[:], x_d[t0:t0 + 128, :])
                xpre[(b, j)] = xt

        # second wave: FFN weights (queued behind phase-A essentials)
        for d in range(N_D):
            nc.sync.dma_start(w1_sb[d][:], w1_d[128 * d:128 * (d + 1), :])
        for f in range(N_F):
            nc.sync.dma_start(w2_sb[f][:], w2_d[128 * f:128 * (f + 1), :])

        dpool = ctx.enter_context(tc.tile_pool(name="dram", bufs=1, space="DRAM"))
        chain = [dpool.tile([NTOK, D], F32, tag=f"chain{i}", name=f"chain{i}")
                 for i in range(max(n_reps - 1, 0))]


        # PSUM budget (8 banks): sc 2x3 + shared proj/o 2x1
        ps_sc = ctx.enter_context(tc.tile_pool(name="ps_sc", bufs=2, space="PSUM"))
        ps_po = ctx.enter_context(tc.tile_pool(name="ps_po", bufs=2, space="PSUM"))

        def layernorm_tiles(src_tiles, rs_out=None, nm_out=None):
            """LN over 4 token tiles; returns 4 normalized bf16 tiles.
            rsqrt computed as exp(-0.5*ln(var+eps)) to stay on ACT set
            {ln,exp}. If rs_out/nm_out given, only computes the scales
            (phase-A LN2 stats) and returns None."""
            mv = stpool.tile([128, 8], F32, tag="mv", name="mv")
            for j in range(4):
                st6 = stpool.tile([128, 6], F32, tag="st6", name="st6")
                nc.vector.bn_stats(st6[:], src_tiles[j][:])
                nc.vector.bn_aggr(mv[:, 2 * j:2 * j + 2], st6[:])
            rs = rs_out if rs_out is not None else stpool.tile([128, 4], F32, tag="rs", name="rs")
            nmr = nm_out if nm_out is not None else stpool.tile([128, 4], F32, tag="nmr", name="nmr")
            mv3 = mv[:].rearrange("p (j two) -> p j two", two=2)
            # rsqrt(var+eps) on DVE only: fast-inverse-sqrt seed + 2 Newton
            # steps (keeps the ACT table on {exp}/{gelu} all rep long)
            u = stpool.tile([128, 4], F32, tag="u", name="u")
            vh = stpool.tile([128, 4], F32, tag="vh", name="vh")
            yy = stpool.tile([128, 4], F32, tag="yy", name="yy")
            nc.vector.tensor_scalar(u[:], mv3[:, :, 1], EPS, None, op0=ALU.add)
            nc.vector.tensor_scalar(vh[:], u[:], 0.5, None, op0=ALU.mult)
            ui = u[:].bitcast(mybir.dt.uint32)
            nc.vector.tensor_scalar(ui, ui, 1, None, op0=ALU.logical_shift_right)
            nc.vector.tensor_tensor(rs[:].bitcast(mybir.dt.uint32), magic_sb[:],
                                    ui, op=ALU.subtract)
            for _ in range(2):
                nc.vector.tensor_mul(yy[:], rs[:], rs[:])
                nc.vector.tensor_mul(yy[:], yy[:], vh[:])
                nc.vector.tensor_scalar(yy[:], yy[:], -1.0, 1.5,
                                        op0=ALU.mult, op1=ALU.add)
                nc.vector.tensor_mul(rs[:], rs[:], yy[:])
            for j in range(4):
                # nmr = -(mu * rsig)
                nc.vector.tensor_scalar(nmr[:, j:j + 1], mv3[:, j, 0].unsqueeze(-1),
                                        rs[:, j:j + 1], -1.0,
                                        op0=ALU.mult, op1=ALU.mult)
            if rs_out is not None:
                return None
            xn_tiles = []
            for j in range(4):
                xn = xnpool.tile([128, D], BF16, tag="xn", name="xn")
                nc.vector.tensor_scalar(xn[:], src_tiles[j][:], rs[:, j:j + 1],
                                        nmr[:, j:j + 1], op0=ALU.mult, op1=ALU.add)
                xn_tiles.append(xn)
            return xn_tiles

        def apply_ln(src_tiles, rs, nmr):
            xn_tiles = []
            for j in range(4):
                xn = xnpool.tile([128, D], BF16, tag="xn", name="xn")
                nc.vector.tensor_scalar(xn[:], src_tiles[j][:], rs[:, j:j + 1],
                                        nmr[:, j:j + 1], op0=ALU.mult, op1=ALU.add)
                xn_tiles.append(xn)
            return xn_tiles

        def transpose_to_feature_major(xn_tiles):
            """4x [128, D] token-major bf16 -> 3x [128, 512] feature-major."""
            xT = []
            for d in range(N_D):
                ps = ps_po.tile([128, 512], F32, tag="po", name="ps_t")
                psb = ps[:].bitcast(BF16)   # transpose out must match in dtype
                for j in range(4):
                    nc.tensor.transpose(
                        psb[:, 128 * j:128 * (j + 1)],
                        xn_tiles[j][:, 128 * d:128 * (d + 1)],
                        identity[:],
                    )
                t = xTpool.tile([128, 512], BF16, tag=f"xT{d}", name=f"xT{d}")
                nc.vector.tensor_copy(t[:], psb[:, 0:512])
                xT.append(t)
            return xT

        # ======================= per-rep =======================
        for rep in range(n_reps):
          xsrc_d = x_d if rep == 0 else chain[rep - 1]
          xdst_d = out_d if rep == n_reps - 1 else chain[rep]

          # ------------------- phase A: attention -------------------
          for b in range(n_b):
            xin = []
            for j in range(4):
                t0 = 128 * (4 * b + j)
                if rep == 0 and (b, j) in xpre:
                    xin.append(xpre[(b, j)])
                    continue
                xt = xpool.tile([128, D], F32, tag="x", name="xt")
                nc.sync.dma_start(xt[:], xsrc_d[t0:t0 + 128, :])
                xin.append(xt)
            xn_tiles = layernorm_tiles(xin)
            xT = transpose_to_feature_major(xn_tiles)

            # Q^T / K^T per head-pair: [54, 512] at partitions 0-53/64-117
            qt, kt = [], []
            for p in range(PAIRS):
                m = 118 if p < PAIRS - 1 else 54
                for (dst_list, w_sb, tg) in ((qt, wq_sb, "q"), (kt, wk_sb, "k")):
                    ps = ps_po.tile([128, 512], F32, tag="po", name="ps_qk")
                    for d in range(N_D):
                        nc.tensor.matmul(
                            ps[0:m, :],
                            w_sb[d][:, 128 * p:128 * p + m],
                            xT[d][:],
                            start=(d == 0), stop=(d == N_D - 1),
                        )
                    t = qkpool.tile([128, 512], BF16, tag=f"{tg}{p}", name=f"{tg}{p}")
                    nc.scalar.copy(t[0:m, :], ps[0:m, :])
                    dst_list.append(t)

            # V token-major with a ones column per head
            vt = []
            for j in range(4):
                ps = ps_po.tile([128, 512], F32, tag="po", name="ps_v")
                for d in range(N_D):
                    nc.tensor.matmul(
                        ps[:],
                        xT[d][:, 128 * j:128 * (j + 1)],
                        wv_sb[d][:],
                        start=(d == 0), stop=(d == N_D - 1),
                    )
                t = vpool.tile([128, 512], BF16, tag="v", name="vt")
                nc.gpsimd.memset(t[:], 1.0)
                src = ps[:, 0:448].rearrange("p (h c) -> p h c", h=H)[:, :, 1:55]
                dst = t[:, 0:448].rearrange("p (h c) -> p h c", h=H)[:, :, 1:55]
                nc.scalar.copy(dst, src)
                vt.append(t)

            # ---- per-head attention ----
            ot_b = [None] * H
            for h in range(H):
                p, sl = h // 2, 64 * (h % 2)
                sc = ps_sc.tile([128, 1536], F32, tag="sc", name="sc")
                for j in range(4):
                    # shared stationary kt chunk: diagonal block then rect
                    nc.tensor.matmul(
                        sc[:, 128 * j:128 * (j + 1)],
                        kt[p][sl:sl + HS, 128 * j:128 * (j + 1)],
                        qt[p][sl:sl + HS, 128 * j:128 * (j + 1)],
                        start=True, stop=True,
                    )
                    if j < 3:
                        nc.tensor.matmul(
                            sc[:, R_OFF[j]:R_OFF[j] + R_W[j]],
                            kt[p][sl:sl + HS, 128 * j:128 * (j + 1)],
                            qt[p][sl:sl + HS, 128 * (j + 1):512],
                            start=True, stop=True,
                        )
                eT = epool.tile([128, 1280], BF16, tag="eT", name="eT")
                nc.scalar.activation(eT[:, 0:EXP_END], sc[:, 0:EXP_END], AF.Exp)
                # causal mask on the packed diagonal blocks (one DVE op)
                nc.vector.tensor_mul(eT[:, 0:512], eT[:, 0:512], tri4[:])
                # o^T accumulate; row 0 = softmax denominator
                ops = ps_po.tile([128, 512], F32, tag="po", name="ops")
                for j in range(3):
                    nc.tensor.matmul(
                        ops[0:55, 128 * (j + 1):512],
                        vt[j][:, 64 * h: 64 * h + 55],
                        eT[:, R_OFF[j]:R_OFF[j] + R_W[j]],
                        start=(j == 0), stop=False,
                    )
                for j in range(4):
                    nc.tensor.matmul(
                        ops[0:55, 128 * j:128 * (j + 1)],
                        vt[j][:, 64 * h: 64 * h + 55],
                        eT[:, 128 * j:128 * (j + 1)],
                        start=False, stop=(j == 3),
                    )
                osb = rpool.tile([55, 512], BF16, tag="osb", name="osb")
                nc.vector.tensor_copy(osb[:], ops[0:55, :])
                rb = rpool.tile([55, 512], BF16, tag="rb", name="rb")
                nc.sync.dma_start(
                    rb[:], osb[0:1, :].unsqueeze(1).to_broadcast([1, 55, 512]))
                ot = otpool.tile([55, 512], BF16, tag=f"ot{h}", name=f"ot{h}")
                nc.vector.tensor_tensor(ot[:], osb[:], rb[:], op=ALU.divide)
                ot_b[h] = ot

            # ---- attention out-proj + residual -> x2 (SBUF) ----
            for j in range(4):
                ps = ps_po.tile([128, 512], F32, tag="po", name="ps_wo")
                for h in range(H):
                    nc.tensor.matmul(
                        ps[:, 0:D],
                        ot_b[h][:, 128 * j:128 * (j + 1)],
                        wo_sb[h][:],
                        start=(h == 0), stop=(h == H - 1 and not has_bias_o),
                    )
                if has_bias_o:
                    nc.tensor.matmul(ps[:, 0:D], ones_sb[:], bo_sb[:],
                                     start=False, stop=True)
                nc.vector.tensor_add(x2_sb[b][j][:], ps[:, 0:D], xin[j][:])
            # LN2 stats for this batch (ACT still on {ln,exp})
            layernorm_tiles([x2_sb[b][j] for j in range(4)],
                            rs_out=ln2_rs[b], nm_out=ln2_nm[b])

          # ------------------- phase B: FFN -------------------
          for b in range(n_b):
            xn2 = apply_ln([x2_sb[b][j] for j in range(4)],
                           ln2_rs[b], ln2_nm[b])
            xT2 = transpose_to_feature_major(xn2)

            # FFN2 accumulators: 4 token-chunks in the 2 sc psum tiles
            # (bitcast to f32: [128, 1024] = 2 banks; chunks at 0 and 512)
            acc_t = [ps_sc.tile([128, 1536], F32, tag="sc", name="acc")
                     for _ in range(2)]
            acc = [acc_t[j // 2][:, 512 * (j % 2): 512 * (j % 2) + D]
                   for j in range(4)]
            for f in range(N_F):
                ps = ps_po.tile([128, 512], F32, tag="po", name="ps_f1")
                for d in range(N_D):
                    nc.tensor.matmul(
                        ps[:],
                        w1_sb[d][:, 128 * f:128 * (f + 1)],
                        xT2[d][:],
                        start=(d == 0), stop=(d == N_D - 1),
                    )
                hg = hpool.tile([128, 512], BF16, tag="hg", name="hg")
                nc.scalar.activation(hg[:], ps[:], AF.Gelu, bias=b1_sb[:, f:f + 1])
                for j in range(4):
                    nc.tensor.matmul(
                        acc[j],
                        hg[:, 128 * j:128 * (j + 1)],
                        w2_sb[f][:],
                        start=(f == 0),
                        stop=(f == N_F - 1 and not has_bias_2),
                    )
            for j in range(4):
                t0 = 128 * (4 * b + j)
                if has_bias_2:
                    nc.tensor.matmul(acc[j], ones_sb[:], b2_sb[:],
                                     start=False, stop=True)
                ot = opool.tile([128, D], F32, tag="out", name="outt")
                nc.vector.tensor_add(ot[:], acc[j], x2_sb[b][j][:])
                nc.sync.dma_start(xdst_d[t0:t0 + 128, :], ot[:])

    nc.finalize()
    return nc


def preprocess(wq, bq, wk, bk, wv, bv, wo, bo, w1, b1, w2, b2,
               ln1_g, ln1_b, ln2_g, ln2_b):
    """Host-side folding: LN affine into weight matrices, attention scale into
    Q, V-bias into output bias; build padded/packed bf16 layouts."""
    import ml_dtypes
    f32 = np.float32
    bf16 = ml_dtypes.bfloat16
    args = [np.asarray(a, f32) for a in (wq, bq, wk, bk, wv, bv, wo, bo,
                                         w1, b1, w2, b2, ln1_g, ln1_b, ln2_g, ln2_b)]
    (wq, bq, wk, bk, wv, bv, wo, bo, w1, b1, w2, b2,
     ln1_g, ln1_b, ln2_g, ln2_b) = args
    scale = f32(HS) ** f32(-0.5)

    wq_pad = np.zeros((D, 512), f32)
    wk_pad = np.zeros((D, 512), f32)
    wv_pad = np.zeros((D, 512), f32)
    for h in range(H):
        wq_pad[:, 64 * h:64 * h + HS] = ln1_g[:, None] * wq[h] * scale
        wk_pad[:, 64 * h:64 * h + HS] = ln1_g[:, None] * wk[h]
        wv_pad[:, 64 * h + 1:64 * h + 1 + HS] = ln1_g[:, None] * wv[h]

    bq_eff = (bq + ln1_b @ wq).astype(f32)     # [H, HS]
    assert not np.any(bq_eff), "nonzero effective q bias not supported"
    # bk_eff shifts scores by a per-s constant -> cancelled by softmax; drop.

    bv_eff = (bv + ln1_b @ wv).astype(f32)     # [H, HS] -> folds into bo
    bo_eff = (bo + bv_eff.reshape(-1) @ wo).astype(f32)

    wo_pad = np.zeros((H, 55, D), f32)
    for h in range(H):
        wo_pad[h, 1:55, :] = wo[54 * h:54 * h + HS, :]

    w1_eff = (ln2_g[:, None] * w1).astype(f32)
    b1_eff = (b1 + ln2_b @ w1).astype(f32)
    b1c = np.ascontiguousarray(b1_eff.reshape(N_F, 128).T)   # [128, 18]

    def bf(a):
        return np.ascontiguousarray(a).astype(bf16)

    return dict(
        wq_pad=bf(wq_pad), wk_pad=bf(wk_pad), wv_pad=bf(wv_pad),
        wo_pad=bf(wo_pad),
        w1=bf(w1_eff), b1c=b1c, w2=bf(w2),
        bo=bf(bo_eff.reshape(1, D)), b2=bf(b2.reshape(1, D)),
        has_bias_o=bool(np.any(bo_eff)), has_bias_2=bool(np.any(b2)),
    )


def kernel(**inputs):
    x = np.asarray(inputs["x"], np.float32)
    w = preprocess(
        inputs["wq"], inputs["bq"], inputs["wk"], inputs["bk"],
        inputs["wv"], inputs["bv"], inputs["wo"], inputs["bo"],
        inputs["w1"], inputs["b1"], inputs["w2"], inputs["b2"],
        inputs["ln1_g"], inputs["ln1_b"], inputs["ln2_g"], inputs["ln2_b"],
    )
    has_bo, has_b2 = w.pop("has_bias_o"), w.pop("has_bias_2")
    nc = build_program(n_b=B_LOC, has_bias_o=has_bo, has_bias_2=has_b2)

    core_ids = list(range(N_CORES))
    in_maps = []
    for c in core_ids:
        m = dict(w)
        m["x"] = np.ascontiguousarray(
            x[B_LOC * c:B_LOC * (c + 1)].reshape(B_LOC * S, D))
        in_maps.append(m)

    res = run_bass_kernel_spmd(nc, in_maps, core_ids)
    global LAST_RESULTS
    LAST_RESULTS = res
    out = np.concatenate(
        [res.results[i]["out"].reshape(B_LOC, S, D) for i in range(N_CORES)], axis=0
    )
    return out.astype(np.float32)


# revision 18
# speedup vs baseline: 1.5295x; 1.1759x over previous
"""Trainium2 Bass kernel for a pre-LN transformer block (causal MHA + GELU FFN).

Problem: x[64, 512, 384], 7 heads x 54, FFN 2304. Sharded data-parallel over
batch across 8 NeuronCores (8 batches/core); no collectives needed.

v1 (bf16 rewrite of the fp32r baseline):
  - all matmul operands bf16 (fp32 accumulate in PSUM); rel-err budget 2e-2
  - LN rsqrt via exp(-0.5*ln(var+eps)) so the ACT engine table set stays on
    {ln,exp} for the whole attention phase and {gelu} for the FFN phase
    (2 table loads per rep instead of ~3 per batch)
  - per-head scores live in a 2-bank bf16 PSUM tile packed causally:
    4 diagonal 128-blocks at cols 0..512, rectangles at 512..896 /
    1024..1280 / 1280..1408; one exp() call [0..1408]; one 4-block
    triangular mask multiply on the diagonal region
  - x2 (post-attention residual) persists in SBUF; x is loaded once
  - FFN2 accumulates into the (phase-A scores) PSUM tiles bitcast to f32
"""

import numpy as np
from contextlib import ExitStack

import concourse.bass as bass
import concourse.bacc as bacc
import concourse.mybir as mybir
import concourse.tile as tile
from concourse import masks
from concourse.bass_utils import run_bass_kernel_spmd

# ---- problem constants (hardcoded per harness contract) ----
B, S, D = 64, 512, 384
H, HS = 7, 54
FFN = 6 * D  # 2304
EPS = 1e-5
N_CORES = 8
B_LOC = B // N_CORES          # 8 batches per core
F32 = mybir.dt.float32
BF16 = mybir.dt.bfloat16
AF = mybir.ActivationFunctionType
ALU = mybir.AluOpType

N_D = D // 128                # 3 d-chunks
N_F = FFN // 128              # 18 ffn-chunks
PAIRS = (H + 1) // 2          # 4 head-pair groups (last has 1 head)

# packed causal-score layout inside a [128, 1536] f32 PSUM tile (3 banks;
# f32 bank = 512 elements -> matmul outputs must not cross 512-col lines).
# chunk j holds scores for key-block j vs queries s in [128j, 512) --
# width 512-128j, diagonal 128-block at the chunk start:
#   j0 [0..512) bank0 | j1 [512..896) + j3 [896..1024) bank1 | j2 [1024..1280) bank2
P_OFF = (0, 512, 1024, 896)
P_W = (512, 384, 256, 128)
EXP_END = 1280                # exp() covers [0, 1280) in one call

LAST_RESULTS = None


def build_program(n_b=B_LOC, has_bias_o=False, has_bias_2=False, n_reps=1):
    nc = bacc.Bacc()
    NTOK = n_b * S

    x_d = nc.declare_dram_parameter("x", [NTOK, D], F32, isOutput=False)
    wq_d = nc.declare_dram_parameter("wq_pad", [D, 512], BF16, isOutput=False)
    wk_d = nc.declare_dram_parameter("wk_pad", [D, 512], BF16, isOutput=False)
    wv_d = nc.declare_dram_parameter("wv_pad", [D, 512], BF16, isOutput=False)
    wo_d = nc.declare_dram_parameter("wo_pad", [H, 55, D], BF16, isOutput=False)
    w1_d = nc.declare_dram_parameter("w1", [D, FFN], BF16, isOutput=False)
    w2_d = nc.declare_dram_parameter("w2", [FFN, D], BF16, isOutput=False)
    b1_d = nc.declare_dram_parameter("b1c", [128, N_F], F32, isOutput=False)
    bo_d = nc.declare_dram_parameter("bo", [1, D], BF16, isOutput=False)
    b2_d = nc.declare_dram_parameter("b2", [1, D], BF16, isOutput=False)
    out_d = nc.declare_dram_parameter("out", [NTOK, D], F32, isOutput=True)

    with tile.TileContext(nc) as tc, ExitStack() as ctx, \
            nc.allow_low_precision(reason="bf16 kernel; rel-err gate 2e-2"):
        # ---------------- persistent pools ----------------
        wpool = ctx.enter_context(tc.tile_pool(name="weights", bufs=1))
        wq_sb = [wpool.tile([128, 512], BF16, tag=f"wq{d}", name=f"wq{d}") for d in range(N_D)]
        wk_sb = [wpool.tile([128, 512], BF16, tag=f"wk{d}", name=f"wk{d}") for d in range(N_D)]
        wv_sb = [wpool.tile([128, 512], BF16, tag=f"wv{d}", name=f"wv{d}") for d in range(N_D)]
        wo_sb = [wpool.tile([55, D], BF16, tag=f"wo{h}", name=f"wo{h}") for h in range(H)]
        w1_sb = [wpool.tile([128, FFN], BF16, tag=f"w1{d}", name=f"w1{d}") for d in range(N_D)]
        w2_sb = [wpool.tile([128, D], BF16, tag=f"w2{f}", name=f"w2{f}") for f in range(N_F)]
        b1_sb = wpool.tile([128, N_F], F32, tag="b1")
        bo_sb = wpool.tile([1, D], BF16, tag="bo")
        b2_sb = wpool.tile([1, D], BF16, tag="b2")
        eps_sb = wpool.tile([128, 1], F32, tag="eps")
        magic_sb = wpool.tile([128, 4], mybir.dt.uint32, tag="magic")
        ones_sb = wpool.tile([1, 128], BF16, tag="ones")
        tri4 = wpool.tile([128, 512], BF16, tag="tri4")
        identity = wpool.tile([128, 128], BF16, tag="ident")

        # x2 persists in SBUF between the attention and FFN phases
        x2pool = ctx.enter_context(tc.tile_pool(name="x2", bufs=1))
        x2_sb = [[x2pool.tile([128, D], F32, tag=f"x2_{b}_{j}", name=f"x2_{b}_{j}")
                  for j in range(4)] for b in range(n_b)]
        ln2_rs = [x2pool.tile([128, 4], F32, tag=f"rs2_{b}", name=f"rs2_{b}") for b in range(n_b)]
        ln2_nm = [x2pool.tile([128, 4], F32, tag=f"nm2_{b}", name=f"nm2_{b}") for b in range(n_b)]

        # ---------------- streaming pools ----------------
        xpool = ctx.enter_context(tc.tile_pool(name="xin", bufs=12))
        stpool = ctx.enter_context(tc.tile_pool(name="stats", bufs=4))
        xnpool = ctx.enter_context(tc.tile_pool(name="xn", bufs=8))
        xTpool = ctx.enter_context(tc.tile_pool(name="xT", bufs=2))
        qkpool = ctx.enter_context(tc.tile_pool(name="qk", bufs=2))
        vpool = ctx.enter_context(tc.tile_pool(name="v", bufs=8))
        epool = ctx.enter_context(tc.tile_pool(name="expT", bufs=3))
        rpool = ctx.enter_context(tc.tile_pool(name="recip", bufs=3))
        otpool = ctx.enter_context(tc.tile_pool(name="oT", bufs=2))
        hpool = ctx.enter_context(tc.tile_pool(name="hgelu", bufs=4))
        opool = ctx.enter_context(tc.tile_pool(name="outt", bufs=4))

        # prologue DMAs: batch-0 x first, then phase-A weights
        xpre = {}
        for j in range(4):
            xt = xpool.tile([128, D], F32, tag="x", name="xt")
            nc.sync.dma_start(xt[:], x_d[128 * j:128 * (j + 1), :])
            xpre[(0, j)] = xt
        for d in range(N_D):
            nc.sync.dma_start(wq_sb[d][:], wq_d[128 * d:128 * (d + 1), :])
            nc.sync.dma_start(wk_sb[d][:], wk_d[128 * d:128 * (d + 1), :])
            nc.sync.dma_start(wv_sb[d][:], wv_d[128 * d:128 * (d + 1), :])
        for h in range(H):
            nc.sync.dma_start(wo_sb[h][:], wo_d[h])
        nc.sync.dma_start(b1_sb[:], b1_d[:])
        nc.sync.dma_start(bo_sb[:], bo_d[:])
        nc.sync.dma_start(b2_sb[:], b2_d[:])
        nc.any.memset(eps_sb[:], EPS)
        nc.any.memset(magic_sb[:].bitcast(F32), np.uint32(0x5F3759DF).view(np.float32))
        nc.any.memset(ones_sb[:], 1.0)
        masks.make_identity(nc, identity[:])
        for j in range(4):
            masks.make_upper_triangular(nc, tri4[:, 128 * j:128 * (j + 1)],
                                        val=1.0, diag=True)


        # prefetch batch-1 x ahead of the FFN weights
        if n_b > 1:
            for j in range(4):
                t0 = 128 * (4 + j)
                xt = xpool.tile([128, D], F32, tag="x", name="xt")
                nc.sync.dma_start(xt[:], x_d[t0:t0 + 128, :])
                xpre[(1, j)] = xt

        # second wave: FFN weights (queued behind phase-A essentials)
        for d in range(N_D):
            nc.sync.dma_start(w1_sb[d][:], w1_d[128 * d:128 * (d + 1), :])
        for f in range(N_F):
            nc.sync.dma_start(w2_sb[f][:], w2_d[128 * f:128 * (f + 1), :])

        dpool = ctx.enter_context(tc.tile_pool(name="dram", bufs=1, space="DRAM"))
        chain = [dpool.tile([NTOK, D], F32, tag=f"chain{i}", name=f"chain{i}")
                 for i in range(max(n_reps - 1, 0))]


        # PSUM budget (8 banks): sc 2x3 + shared proj/o 2x1
        ps_sc = ctx.enter_context(tc.tile_pool(name="ps_sc", bufs=2, space="PSUM"))
        ps_po = ctx.enter_context(tc.tile_pool(name="ps_po", bufs=2, space="PSUM"))

        def ln_stats(src_tiles, rs, nmr):
            """LN scale/shift for 4 token tiles -> rs/nmr [128, 4].
            rsqrt(var+eps) entirely on DVE (bit-trick seed + 2 Newton
            steps) so the ACT table set never changes mid-phase."""
            mv = stpool.tile([128, 8], F32, tag="mv", name="mv")
            for j in range(4):
                st6 = stpool.tile([128, 6], F32, tag="st6", name="st6")
                nc.vector.bn_stats(st6[:], src_tiles[j][:])
                nc.vector.bn_aggr(mv[:, 2 * j:2 * j + 2], st6[:])
            mv3 = mv[:].rearrange("p (j two) -> p j two", two=2)
            # rsqrt(var+eps) on DVE only: fast-inverse-sqrt seed + 2 Newton
            # steps (keeps the ACT table on {exp}/{gelu} all rep long)
            u = stpool.tile([128, 4], F32, tag="u", name="u")
            vh = stpool.tile([128, 4], F32, tag="vh", name="vh")
            yy = stpool.tile([128, 4], F32, tag="yy", name="yy")
            nc.vector.tensor_scalar(u[:], mv3[:, :, 1], EPS, None, op0=ALU.add)
            nc.vector.tensor_scalar(vh[:], u[:], 0.5, None, op0=ALU.mult)
            ui = u[:].bitcast(mybir.dt.uint32)
            nc.vector.tensor_scalar(ui, ui, 1, None, op0=ALU.logical_shift_right)
            nc.vector.tensor_tensor(rs[:].bitcast(mybir.dt.uint32), magic_sb[:],
                                    ui, op=ALU.subtract)
            for _ in range(2):
                nc.vector.tensor_mul(yy[:], rs[:], rs[:])
                nc.vector.tensor_mul(yy[:], yy[:], vh[:])
                nc.vector.tensor_scalar(yy[:], yy[:], -1.0, 1.5,
                                        op0=ALU.mult, op1=ALU.add)
                nc.vector.tensor_mul(rs[:], rs[:], yy[:])
            for j in range(4):
                # nmr = -(mu * rsig)
                nc.vector.tensor_scalar(nmr[:, j:j + 1], mv3[:, j, 0].unsqueeze(-1),
                                        rs[:, j:j + 1], -1.0,
                                        op0=ALU.mult, op1=ALU.mult)

        def apply_ln(src_tiles, rs, nmr):
            """xn = x*rs + nmr on ACT (Identity is in every table set)."""
            xn_tiles = []
            for j in range(4):
                xn = xnpool.tile([128, D], BF16, tag="xn", name="xn")
                nc.scalar.activation(xn[:], src_tiles[j][:], AF.Identity,
                                     bias=nmr[:, j:j + 1], scale=rs[:, j:j + 1])
                xn_tiles.append(xn)
            return xn_tiles

        def transpose_to_feature_major(xn_tiles, evac="dve"):
            """4x [128, D] token-major bf16 -> 3x [128, 512] feature-major."""
            xT = []
            for d in range(N_D):
                ps = ps_po.tile([128, 512], F32, tag="po", name="ps_t")
                psb = ps[:].bitcast(BF16)   # transpose out must match in dtype
                for j in range(4):
                    nc.tensor.transpose(
                        psb[:, 128 * j:128 * (j + 1)],
                        xn_tiles[j][:, 128 * d:128 * (d + 1)],
                        identity[:],
                    )
                t = xTpool.tile([128, 512], BF16, tag=f"xT{d}", name=f"xT{d}")
                if evac == "dve":
                    nc.vector.tensor_copy(t[:], psb[:, 0:512])
                else:
                    nc.scalar.copy(t[:], psb[:, 0:512])
                xT.append(t)
            return xT

        # ======================= per-rep =======================
        for rep in range(n_reps):
          xsrc_d = x_d if rep == 0 else chain[rep - 1]
          xdst_d = out_d if rep == n_reps - 1 else chain[rep]

          # ------------------- phase A: attention -------------------
          def load_x(b):
              xin = []
              for j in range(4):
                  t0 = 128 * (4 * b + j)
                  if rep == 0 and (b, j) in xpre:
                      xin.append(xpre[(b, j)])
                      continue
                  xt = xpool.tile([128, D], F32, tag="x", name="xt")
                  nc.sync.dma_start(xt[:], xsrc_d[t0:t0 + 128, :])
                  xin.append(xt)
              return xin

          def ln1_stats(xin):
              rs = stpool.tile([128, 4], F32, tag="rs", name="rs")
              nmr = stpool.tile([128, 4], F32, tag="nmr", name="nmr")
              ln_stats(xin, rs, nmr)
              return rs, nmr

          xin_b = load_x(0)
          rs_b, nmr_b = ln1_stats(xin_b)
          xn_b = apply_ln(xin_b, rs_b, nmr_b)
          for b in range(n_b):
            xin, xn_tiles = xin_b, xn_b
            xT = transpose_to_feature_major(xn_tiles)

            # Q^T / K^T per head-pair: [54, 512] at partitions 0-53/64-117
            qt, kt = [], []
            for p in range(PAIRS):
                m = 118 if p < PAIRS - 1 else 54
                for (dst_list, w_sb, tg) in ((qt, wq_sb, "q"), (kt, wk_sb, "k")):
                    ps = ps_po.tile([128, 512], F32, tag="po", name="ps_qk")
                    for d in range(N_D):
                        nc.tensor.matmul(
                            ps[0:m, :],
                            w_sb[d][:, 128 * p:128 * p + m],
                            xT[d][:],
                            start=(d == 0), stop=(d == N_D - 1),
                        )
                    t = qkpool.tile([128, 512], BF16, tag=f"{tg}{p}", name=f"{tg}{p}")
                    if tg == "q":
                        nc.scalar.copy(t[0:m, :], ps[0:m, :])
                    else:
                        nc.vector.tensor_copy(t[0:m, :], ps[0:m, :])
                    dst_list.append(t)

            # V token-major with a ones column per head
            vt = []
            for j in range(4):
                ps = ps_po.tile([128, 512], F32, tag="po", name="ps_v")
                for d in range(N_D):
                    nc.tensor.matmul(
                        ps[:],
                        xT[d][:, 128 * j:128 * (j + 1)],
                        wv_sb[d][:],
                        start=(d == 0), stop=(d == N_D - 1),
                    )
                t = vpool.tile([128, 512], BF16, tag="v", name="vt")
                nc.gpsimd.memset(t[:], 1.0)
                src = ps[:, 0:448].rearrange("p (h c) -> p h c", h=H)[:, :, 1:55]
                dst = t[:, 0:448].rearrange("p (h c) -> p h c", h=H)[:, :, 1:55]
                nc.scalar.copy(dst, src)
                vt.append(t)

            # next batch's LN1 stats: DVE is idle during this batch's heads
            if b + 1 < n_b:
                xin_b = load_x(b + 1)
                rs_b, nmr_b = ln1_stats(xin_b)

            # ---- per-head attention, software-pipelined on PE:
            #   iter h emits: scores(h) | rect-o(h-1) | diag-o(h-2)
            # so the Pool-mask latency never blocks the PE stream.
            ot_b = [None] * H
            eT_b = [None] * H
            ops_b = [None] * H

            def emit_scores(h):
                p, sl = h // 2, 64 * (h % 2)
                sc = ps_sc.tile([128, 1536], F32, tag="sc", name="sc")
                for j in range(4):
                    # one MM per key-block: [keys 128j.., queries 128j..512)
                    nc.tensor.matmul(
                        sc[:, P_OFF[j]:P_OFF[j] + P_W[j]],
                        kt[p][sl:sl + HS, 128 * j:128 * (j + 1)],
                        qt[p][sl:sl + HS, 128 * j:512],
                        start=True, stop=True,
                    )
                eT = epool.tile([128, 1280], BF16, tag="eT", name="eT")
                nc.scalar.activation(eT[:, 0:EXP_END], sc[:, 0:EXP_END], AF.Exp)
                # causal mask on the in-chunk diagonal blocks: j0@0, j1@512
                # (stride 512), then j3@896 + j2@1024 (contiguous 256)
                m2 = eT[:, 0:640].rearrange("p (a b) -> p a b", b=128)[:, 0::4]
                t2 = tri4[:, 0:256].rearrange("p (a b) -> p a b", b=128)
                nc.gpsimd.tensor_mul(m2, m2, t2)
                nc.gpsimd.tensor_mul(eT[:, 896:1152], eT[:, 896:1152],
                                     tri4[:, 0:256])
                eT_b[h] = eT

            def emit_rect_o(h):
                eT = eT_b[h]
                ops = ps_po.tile([128, 512], F32, tag="po", name="ops")
                ops_b[h] = ops

            def emit_diag_o(h):
                eT, ops = eT_b[h], ops_b[h]
                for j in range(4):
                    nc.tensor.matmul(
                        ops[0:55, 128 * j:512],
                        vt[j][:, 64 * h: 64 * h + 55],
                        eT[:, P_OFF[j]:P_OFF[j] + P_W[j]],
                        start=(j == 0), stop=(j == 3),
                    )
                osb = rpool.tile([55, 512], BF16, tag="osb", name="osb")
                nc.vector.tensor_copy(osb[:], ops[0:55, :])
                ot = otpool.tile([55, 512], BF16, tag=f"ot{h}", name=f"ot{h}")
                if h == H - 1:
                    # last head gates Wo: broadcast the denominator row via a
                    # PE outer-product (fast) instead of the DMA broadcast
                    rbp = ps_sc.tile([128, 1536], F32, tag="sc", name="rb_ps")
                    nc.tensor.matmul(rbp[0:55, 0:512], ones_sb[0:1, 0:55],
                                     osb[0:1, :], start=True, stop=True)
                    nc.vector.tensor_tensor(ot[:], osb[:], rbp[0:55, 0:512],
                                            op=ALU.divide)
                else:
                    rb = rpool.tile([55, 512], BF16, tag="rb", name="rb")
                    nc.sync.dma_start(
                        rb[:], osb[0:1, :].unsqueeze(1).to_broadcast([1, 55, 512]))
                    nc.vector.tensor_tensor(ot[:], osb[:], rb[:], op=ALU.divide)
                ot_b[h] = ot

            for h in range(H + 2):
                if h < H:
                    emit_scores(h)
                if 1 <= h <= H:
                    emit_rect_o(h - 1)
                if 2 <= h <= H + 1:
                    emit_diag_o(h - 2)

            # next batch's LN1 apply: ACT is idle during Wo
            if b + 1 < n_b:
                xn_b = apply_ln(xin_b, rs_b, nmr_b)

            # ---- attention out-proj + residual -> x2 (SBUF) ----
            for j in range(4):
                ps = ps_po.tile([128, 512], F32, tag="po", name="ps_wo")
                for h in range(H):
                    nc.tensor.matmul(
                        ps[:, 0:D],
                        ot_b[h][:, 128 * j:128 * (j + 1)],
                        wo_sb[h][:],
                        start=(h == 0), stop=(h == H - 1 and not has_bias_o),
                    )
                if has_bias_o:
                    nc.tensor.matmul(ps[:, 0:D], ones_sb[:], bo_sb[:],
                                     start=False, stop=True)
                nc.vector.tensor_add(x2_sb[b][j][:], ps[:, 0:D], xin[j][:])
            # LN2 scale/shift for this batch (pure DVE)
            ln_stats([x2_sb[b][j] for j in range(4)], ln2_rs[b], ln2_nm[b])

          # ------------------- phase B: FFN -------------------
          xn2_b = apply_ln([x2_sb[0][j] for j in range(4)],
                           ln2_rs[0], ln2_nm[0])
          for b in range(n_b):
            xn2 = xn2_b
            xT2 = transpose_to_feature_major(xn2, evac="act")

            # FFN2 accumulators: 4 token-chunks in the 2 sc psum tiles
            # (bitcast to f32: [128, 1024] = 2 banks; chunks at 0 and 512)
            acc_t = [ps_sc.tile([128, 1536], F32, tag="sc", name="acc")
                     for _ in range(2)]
            acc = [acc_t[j // 2][:, 512 * (j % 2): 512 * (j % 2) + D]
                   for j in range(4)]
            for f in range(N_F):
                ps = ps_po.tile([128, 512], F32, tag="po", name="ps_f1")
                for d in range(N_D):
                    nc.tensor.matmul(
                        ps[:],
                        w1_sb[d][:, 128 * f:128 * (f + 1)],
                        xT2[d][:],
                        start=(d == 0), stop=(d == N_D - 1),
                    )
                hg = hpool.tile([128, 512], BF16, tag="hg", name="hg")
                nc.scalar.activation(hg[:], ps[:], AF.Gelu, bias=b1_sb[:, f:f + 1])
                if f == 5 and b + 1 < n_b:
                    xn2_b = apply_ln([x2_sb[b + 1][j] for j in range(4)],
                                     ln2_rs[b + 1], ln2_nm[b + 1])
                for j in range(4):
                    nc.tensor.matmul(
                        acc[j],
                        hg[:, 128 * j:128 * (j + 1)],
                        w2_sb[f][:],
                        start=(f == 0),
                        stop=(f == N_F - 1 and not has_bias_2),
                    )
            for j in range(4):
                t0 = 128 * (4 * b + j)
                if has_bias_2:
                    nc.tensor.matmul(acc[j], ones_sb[:], b2_sb[:],
                                     start=False, stop=True)
                ot = opool.tile([128, D], F32, tag="out", name="outt")
                nc.vector.tensor_add(ot[:], acc[j], x2_sb[b][j][:])
                nc.sync.dma_start(xdst_d[t0:t0 + 128, :], ot[:])

    nc.finalize()
    return nc


def preprocess(wq, bq, wk, bk, wv, bv, wo, bo, w1, b1, w2, b2,
               ln1_g, ln1_b, ln2_g, ln2_b):
    """Host-side folding: LN affine into weight matrices, attention scale into
    Q, V-bias into output bias; build padded/packed bf16 layouts."""
    import ml_dtypes
    f32 = np.float32
    bf16 = ml_dtypes.bfloat16
    args = [np.asarray(a, f32) for a in (wq, bq, wk, bk, wv, bv, wo, bo,
                                         w1, b1, w2, b2, ln1_g, ln1_b, ln2_g, ln2_b)]
    (wq, bq, wk, bk, wv, bv, wo, bo, w1, b1, w2, b2,
     ln1_g, ln1_b, ln2_g, ln2_b) = args
    scale = f32(HS) ** f32(-0.5)

    wq_pad = np.zeros((D, 512), f32)
    wk_pad = np.zeros((D, 512), f32)
    wv_pad = np.zeros((D, 512), f32)
    for h in range(H):
        wq_pad[:, 64 * h:64 * h + HS] = ln1_g[:, None] * wq[h] * scale
        wk_pad[:, 64 * h:64 * h + HS] = ln1_g[:, None] * wk[h]
        wv_pad[:, 64 * h + 1:64 * h + 1 + HS] = ln1_g[:, None] * wv[h]

    bq_eff = (bq + ln1_b @ wq).astype(f32)     # [H, HS]
    assert not np.any(bq_eff), "nonzero effective q bias not supported"
    # bk_eff shifts scores by a per-s constant -> cancelled by softmax; drop.

    bv_eff = (bv + ln1_b @ wv).astype(f32)     # [H, HS] -> folds into bo
    bo_eff = (bo + bv_eff.reshape(-1) @ wo).astype(f32)

    wo_pad = np.zeros((H, 55, D), f32)
    for h in range(H):
        wo_pad[h, 1:55, :] = wo[54 * h:54 * h + HS, :]

    w1_eff = (ln2_g[:, None] * w1).astype(f32)
    b1_eff = (b1 + ln2_b @ w1).astype(f32)
    b1c = np.ascontiguousarray(b1_eff.reshape(N_F, 128).T)   # [128, 18]

    def bf(a):
        return np.ascontiguousarray(a).astype(bf16)

    return dict(
        wq_pad=bf(wq_pad), wk_pad=bf(wk_pad), wv_pad=bf(wv_pad),
        wo_pad=bf(wo_pad),
        w1=bf(w1_eff), b1c=b1c, w2=bf(w2),
        bo=bf(bo_eff.reshape(1, D)), b2=bf(b2.reshape(1, D)),
        has_bias_o=bool(np.any(bo_eff)), has_bias_2=bool(np.any(b2)),
    )


def kernel(**inputs):
    x = np.asarray(inputs["x"], np.float32)
    w = preprocess(
        inputs["wq"], inputs["bq"], inputs["wk"], inputs["bk"],
        inputs["wv"], inputs["bv"], inputs["wo"], inputs["bo"],
        inputs["w1"], inputs["b1"], inputs["w2"], inputs["b2"],
        inputs["ln1_g"], inputs["ln1_b"], inputs["ln2_g"], inputs["ln2_b"],
    )
    has_bo, has_b2 = w.pop("has_bias_o"), w.pop("has_bias_2")
    nc = build_program(n_b=B_LOC, has_bias_o=has_bo, has_bias_2=has_b2)

    core_ids = list(range(N_CORES))
    in_maps = []
    for c in core_ids:
        m = dict(w)
        m["x"] = np.ascontiguousarray(
            x[B_LOC * c:B_LOC * (c + 1)].reshape(B_LOC * S, D))
        in_maps.append(m)

    res = run_bass_kernel_spmd(nc, in_maps, core_ids)
    global LAST_RESULTS
    LAST_RESULTS = res
    out = np.concatenate(
        [res.results[i]["out"].reshape(B_LOC, S, D) for i in range(N_CORES)], axis=0
    )
    return out.astype(np.float32)
